# revision 56
# baseline (speedup 1.0000x reference)
"""Trainium2 Bass kernel for GatedSkipFusion (gate conv -> sigmoid blend ->
4-step LIF -> BatchNorm with training stats).

Self-contained: hardcodes shapes T=4, B=8, C=64, H=W=112; shards batch B
across 8 NeuronCores; BN stats via a 64-float AllReduce.

Math:
  gate = sigmoid(pre); fused = enc + gate*(dec-enc). With h = tanh(pre/2):
  gate = 0.5 + 0.5*h, so fused = enc + 0.5*(1+h)*D, D = dec-enc.
  LIF (tau=2, hard reset, v_th=0.15): v_t = 0.5*v_{t-1}*m_{t-1} + fused_t,
  m = (v < th). Spikes are binary so BN var = mu - mu^2; the BN output is a
  per-channel affine of the sign record sg = Sign(v - th) in {-1,0,1}:
  out = (a/2)*sg + (a/2 + beta - mu*a), a = gamma*rsqrt(var+eps).

Engine split (software-pipelined with per-stage pair lags so no engine
stream head-of-line blocks; the per-core program is then bound by DVE
occupancy ~107us against a 90us DMA floor at 360 GB/s):
  PE    : gate matmuls and D'=0.5*(dec-enc) via +-0.5*I, all fp32r
          (1 cyc/row; exact-enough: identity matmuls add no spike flips)
  Act   : batched tanh from a 4-bank PSUM tile; per-step Sign with
          accumulation for the BN statistics (lagged one pipeline
          iteration so it never paces DVE)
  DVE   : gD=(1+h)*D', F=gD+enc, the LIF reset-mask and v-update stts,
          and the final affine as a 4x-mode fp16 tensor_scalar
  Pool  : second DMA-issue queue (SWDGE) for stats/pass-2 transfers
  fp16 output (halves the output DMA; ~6e-4 systematic error).
"""

import numpy as np

T, B, C, H, W = 4, 8, 64, 112, 112
NPIX = H * W          # 12544
BL = 448              # pixel block (free dim)
NPAIR = NPIX // (2 * BL)   # 14 pairs of blocks
NTILE = NPAIR * T     # 56 (pair,t) tiles
TH = 0.15
EPS = 1e-5
NCORES = 8
N_TOTAL = T * B * NPIX     # 401408 per-channel element count
N_CORE = T * NPIX          # 50176 per-core per-channel count

_cache = {}


def _build(reps=1, use_collective=True, num_devices=NCORES, d_on_pe=True,
           skip=frozenset()):
    import concourse.bass as bass
    import concourse.bacc as bacc
    import concourse.mybir as mybir
    import concourse.tile as tile

    F32 = mybir.dt.float32
    F32R = mybir.dt.float32r
    F16 = mybir.dt.float16
    AF = mybir.ActivationFunctionType
    OP = mybir.AluOpType
    AX = mybir.AxisListType

    nc = bacc.Bacc("TRN2", target_bir_lowering=False, debug=False,
                   enable_asserts=False, num_devices=num_devices)

    # host pre-arranged layout: [pair, partition(p2*64+c), t, x]
    dec_d = nc.dram_tensor("dec", [NPAIR, 128, T, BL], F32R,
                           kind="ExternalInput")
    enc_d = nc.dram_tensor("enc", [NPAIR, 128, T, BL], F32R,
                           kind="ExternalInput")
    # all parameters packed into one tensor: one DMA at startup
    # cols 0:128 wd | 128:256 we | 256:384 idp | 384:512 idm
    # | 512 bgh | 513 nth | 514 gam | 515 bet
    par_d = nc.dram_tensor("par", [128, 516], F32R, kind="ExternalInput")
    out_d = nc.dram_tensor("out", [NPAIR, 128, T, BL], F16,
                           kind="ExternalOutput")

    with tile.TileContext(nc) as tc:
        with tc.tile_pool(name="const", bufs=1) as cp, \
             tc.tile_pool(name="io", bufs=3) as io, \
             tc.tile_pool(name="wk", bufs=3) as wk, \
             tc.tile_pool(name="wkg", bufs=2) as wkg, \
             tc.tile_pool(name="wkf", bufs=3) as wkf, \
             tc.tile_pool(name="vv", bufs=3) as vv, \
             tc.tile_pool(name="sm", bufs=6) as sm, \
             tc.tile_pool(name="ot", bufs=5) as op_, \
             tc.tile_pool(name="ps", bufs=1, space="PSUM") as ps, \
             tc.tile_pool(name="psd", bufs=1, space="PSUM") as psd, \
             tc.tile_pool(name="dram", bufs=2, space="DRAM") as dp:

            par_t = cp.tile([128, 516], F32R)
            nc.sync.dma_start(par_t[:], par_d[:, :])
            wd_t = par_t[:, 0:128]
            we_t = par_t[:, 128:256]
            idp_t = par_t[:, 256:384]
            idm_t = par_t[:, 384:512]
            bgh_t = par_t[:, 512:513].bitcast(F32)
            nth_t = par_t[:, 513:514].bitcast(F32)
            gam_t = par_t[:, 514:515].bitcast(F32)
            bet_t = par_t[:, 515:516].bitcast(F32)

            store = cp.tile([128, NTILE, BL], F16)    # sign record
            scol = cp.tile([128, NTILE], F32)         # per-tile sign sums



            for _rep in range(reps):
                # ---------------- pass 1 (software-pipelined) ----------------
                # Stage lags per emission iteration k:
                #   dma(k); pre/h/D(k-1); gD/F(k-2); lif(k-3); signs(k-4/k-3)
                # Every instruction's inputs were produced in an earlier
                # iteration, so no engine stream ever head-of-line blocks.
                dec4s, enc4s = {}, {}
                h4s, gD4s, F4s = {}, {}, {}
                vps = {}      # pair -> list of v tiles (per t)

                def emit_dma(p):
                    # halves so the first matmuls unblock after 1/4 of the
                    # pair's bytes (shortens pipeline fill)
                    dec4 = io.tile([128, T, BL], F32R)
                    enc4 = io.tile([128, T, BL], F32R)
                    if p == 0:
                        # dec first: the gate matmuls only need dec
                        nc.sync.dma_start(dec4[:, 0:2], dec_d[p, :, 0:2])
                        nc.sync.dma_start(dec4[:, 2:4], dec_d[p, :, 2:4])
                        nc.sync.dma_start(enc4[:, 0:2], enc_d[p, :, 0:2])
                        nc.sync.dma_start(enc4[:, 2:4], enc_d[p, :, 2:4])
                    else:
                        nc.sync.dma_start(dec4[:, 0:2], dec_d[p, :, 0:2])
                        nc.sync.dma_start(enc4[:, 0:2], enc_d[p, :, 0:2])
                        nc.sync.dma_start(dec4[:, 2:4], dec_d[p, :, 2:4])
                        nc.sync.dma_start(enc4[:, 2:4], enc_d[p, :, 2:4])
                    dec4s[p], enc4s[p] = dec4, enc4

                def emit_signs(p):
                    # sign for (p, 1..3) plus (p+1, 0): all deps one iter old
                    for t in range(1, T):
                        if 0 <= p < NPAIR:
                            idx = p * T + t
                            nc.scalar.activation(
                                store[:, idx], vps[p][t], AF.Sign,
                                bias=nth_t, scale=1.0,
                                accum_out=scol[:, idx:idx + 1])
                    q = p + 1
                    if 0 <= q < NPAIR:
                        idx = q * T
                        nc.scalar.activation(
                            store[:, idx], F4s[q][:, 0], AF.Sign,
                            bias=nth_t, scale=1.0,
                            accum_out=scol[:, idx:idx + 1])

                def emit_pre_h(p):
                    dec4, enc4 = dec4s[p], enc4s[p]
                    P4 = ps.tile([128, T, 512], F32)
                    if p == 0:
                        # warm the PE p-state on the param tile while the
                        # first input DMAs stream; the real matmuls below
                        # overwrite these banks (start=True resets PSUM)
                        for w in range(6):
                            nc.tensor.matmul(out=P4[:, w % T, 0:BL],
                                             lhsT=idp_t,
                                             rhs=par_t[:, 0:448],
                                             start=True, stop=True)
                    for t in range(T):
                        nc.tensor.matmul(out=P4[:, t, 0:BL], lhsT=wd_t,
                                         rhs=dec4[:, t], start=True,
                                         stop=False)
                        nc.tensor.matmul(out=P4[:, t, 0:BL], lhsT=we_t,
                                         rhs=enc4[:, t], start=False,
                                         stop=True)
                    h4 = wk.tile([128, T, BL], F32)
                    nc.scalar.activation(h4[:], P4[:, :, 0:BL], AF.Tanh,
                                         bias=bgh_t, scale=0.5)
                    h4s[p] = h4

                def emit_D(p):
                    dec4, enc4 = dec4s[p], enc4s[p]
                    if p < 2:
                        # fill phase: DVE is idle, and this keeps the PE +
                        # PSUM path off the critical startup chain
                        D4t = wk.tile([128, T, BL], F32)
                        nc.vector.tensor_tensor(D4t[:], dec4[:].bitcast(F32),
                                                enc4[:].bitcast(F32),
                                                OP.subtract)
                        return ("sbuf", D4t)
                    D4ap = psd.tile([128, T, 512], F32)
                    for t in range(T):
                        nc.tensor.matmul(out=D4ap[:, t, 0:BL],
                                         lhsT=idp_t, rhs=dec4[:, t],
                                         start=True, stop=False)
                        nc.tensor.matmul(out=D4ap[:, t, 0:BL],
                                         lhsT=idm_t, rhs=enc4[:, t],
                                         start=False, stop=True)
                    return ("psum", D4ap)

                def emit_gD_F(p, D4pack):
                    # gD = (1+h)*0.5*(dec-enc) = sigma(pre)*(dec-enc)
                    kind, D4ap = D4pack
                    if kind == "sbuf":
                        # D unscaled: fold the 0.5 into F's scalar instead
                        D4v, fscale = D4ap[:], 0.5
                    else:
                        D4v, fscale = D4ap[:, :, 0:BL], 1.0
                    gD4 = wkg.tile([128, T, BL], F32)
                    nc.vector.scalar_tensor_tensor(out=gD4[:], in0=h4s[p][:],
                                                   scalar=1.0,
                                                   in1=D4v,
                                                   op0=OP.add, op1=OP.mult)
                    F4 = wkf.tile([128, T, BL], F32)
                    nc.vector.scalar_tensor_tensor(out=F4[:], in0=gD4[:],
                                                   scalar=fscale,
                                                   in1=enc4s[p][:].bitcast(F32),
                                                   op0=OP.mult, op1=OP.add)
                    F4s[p] = F4
                    del h4s[p]
                    del dec4s[p], enc4s[p]

                def emit_lif(p):
                    F4 = F4s[p]
                    vp = F4[:, 0]
                    vlist = [vp]
                    for t in range(T - 1):
                        vrn = vv.tile([128, BL], F32)
                        nc.vector.scalar_tensor_tensor(
                            out=vrn[:], in0=vp, scalar=TH, in1=vp,
                            op0=OP.is_lt, op1=OP.mult)
                        vpt = sm.tile([128, BL], F32)
                        nc.vector.scalar_tensor_tensor(
                            out=vpt[:], in0=vrn[:], scalar=0.5,
                            in1=F4[:, t + 1], op0=OP.mult, op1=OP.add)
                        vp = vpt[:]
                        vlist.append(vp)
                    vps[p] = vlist

                D4prev = {}
                for k in range(NPAIR + 4):
                    if k >= 3:
                        emit_signs(k - 4)   # signs for pair k-4 (t>=1)
                                            # and pair k-3 (t=0)
                    if k < NPAIR:
                        emit_dma(k)
                    if 0 <= k - 1 < NPAIR:
                        emit_pre_h(k - 1)
                        D4prev[k - 1] = emit_D(k - 1)
                    if 0 <= k - 2 < NPAIR:
                        emit_gD_F(k - 2, D4prev.pop(k - 2))
                    if 0 <= k - 3 < NPAIR:
                        emit_lif(k - 3)

                # ---------------- stats ----------------
                # per-channel sign sum: contract the two 64-partition halves
                # of scol with a stacked identity on PE, then reduce tiles
                red = cp.tile([128, 1], F32)
                nc.vector.tensor_reduce(out=red[:], in_=scol[:, 0:NTILE],
                                        axis=AX.X, op=OP.add)
                # swap halves with two parallel SBUF DMAs, then add:
                # s128sum[p] = per-channel total sign sum, duplicated
                red2 = cp.tile([128, 1], F32)
                nc.sync.dma_start(red2[0:64, :], red[64:128, :])
                nc.gpsimd.dma_start(red2[64:128, :], red[0:64, :])
                s128 = cp.tile([128, 1], F32)
                nc.vector.tensor_tensor(s128[:], red[:], red2[:], OP.add)
                mu = cp.tile([128, 1], F32)
                if use_collective:
                    # local spike count = 0.5*sum_sign + N_CORE/2
                    loc = cp.tile([64, 1], F32)
                    nc.vector.tensor_scalar(out=loc[:], in0=s128[0:64, :],
                                            scalar1=0.5,
                                            scalar2=float(N_CORE) / 2.0,
                                            op0=OP.mult, op1=OP.add)
                    cin = dp.tile([64, 1], F32)
                    cout = dp.tile([64, 1], F32)
                    nc.sync.dma_start(cin[:], loc[:])
                    nc.gpsimd.collective_compute(
                        "AllReduce", OP.add,
                        replica_groups=[list(range(num_devices))],
                        ins=[cin.opt()], outs=[cout.opt()])
                    S128 = cp.tile([128, 1], F32)
                    nc.sync.dma_start(S128[0:64, :], cout[:])
                    nc.gpsimd.dma_start(S128[64:128, :], cout[:])
                    nc.vector.tensor_scalar(out=mu[:], in0=S128[:],
                                            scalar1=1.0 / float(N_TOTAL),
                                            scalar2=None, op0=OP.mult)
                else:
                    # mu = ((0.5*sum + N_CORE/2) * NCORES) / N_TOTAL
                    nc.vector.tensor_scalar(
                        out=mu[:], in0=s128[:],
                        scalar1=0.5 * NCORES / float(N_TOTAL),
                        scalar2=N_CORE * 0.5 * NCORES / float(N_TOTAL),
                        op0=OP.mult, op1=OP.add)
                # x = mu*(1-mu) + eps
                m1 = cp.tile([128, 1], F32)
                nc.vector.tensor_scalar(out=m1[:], in0=mu[:], scalar1=-1.0,
                                        scalar2=1.0, op0=OP.mult, op1=OP.add)
                x = cp.tile([128, 1], F32)
                nc.vector.tensor_tensor(x[:], m1[:], mu[:], OP.mult)
                nc.vector.tensor_scalar(out=x[:], in0=x[:], scalar1=EPS,
                                        scalar2=None, op0=OP.add)
                # r = 1/sqrt(x) + one Newton step r *= 1.5-0.5*x*r^2
                sq = cp.tile([128, 1], F32)
                nc.scalar.activation(sq[:], x[:], AF.Sqrt)
                r0 = cp.tile([128, 1], F32)
                nc.vector.reciprocal(r0[:], sq[:])
                e = cp.tile([128, 1], F32)
                nc.vector.tensor_tensor(e[:], r0[:], r0[:], OP.mult)
                nc.vector.tensor_tensor(e[:], e[:], x[:], OP.mult)
                nc.vector.tensor_scalar(out=e[:], in0=e[:], scalar1=-0.5,
                                        scalar2=1.5, op0=OP.mult, op1=OP.add)
                r = cp.tile([128, 1], F32)
                nc.vector.tensor_tensor(r[:], r0[:], e[:], OP.mult)
                # a = gamma*r ; scale = a/2 ; bias = a/2 + beta - mu*a
                a = cp.tile([128, 1], F32)
                nc.vector.tensor_tensor(a[:], gam_t, r[:], OP.mult)
                sc128 = cp.tile([128, 1], F32)
                nc.vector.tensor_scalar(out=sc128[:], in0=a[:], scalar1=0.5,
                                        scalar2=None, op0=OP.mult)
                tmp = cp.tile([128, 1], F32)
                nc.vector.tensor_tensor(tmp[:], mu[:], a[:], OP.mult)
                b0 = cp.tile([128, 1], F32)
                nc.vector.tensor_tensor(b0[:], bet_t, tmp[:], OP.subtract)
                bi128 = cp.tile([128, 1], F32)
                nc.vector.tensor_tensor(bi128[:], sc128[:], b0[:], OP.add)

                # ---------------- pass 2 ----------------
                for pair in range(NPAIR):
                    ot = op_.tile([128, T, BL], F16)
                    nc.vector.tensor_scalar(
                        out=ot[:], in0=store[:, pair * T:(pair + 1) * T, :],
                        scalar1=sc128[:], scalar2=bi128[:],
                        op0=OP.mult, op1=OP.add)
                    eng = (nc.sync, nc.gpsimd, nc.scalar)[pair % 3]
                    eng.dma_start(out_d[pair], ot[:])

    nc.compile()
    return nc


def _prep_host(dec, enc, Wg, bg, gamma, beta):
    Wg = np.asarray(Wg, dtype=np.float32)
    wdT = np.ascontiguousarray(Wg[:, :64].T)   # [k, m] dec-part
    weT = np.ascontiguousarray(Wg[:, 64:].T)   # enc-part
    wd = np.zeros((128, 128), dtype=np.float32)
    we = np.zeros((128, 128), dtype=np.float32)
    wd[:64, :64] = wdT
    wd[64:, 64:] = wdT
    we[:64, :64] = weT
    we[64:, 64:] = weT
    bgh = np.tile(0.5 * np.asarray(bg, np.float32), 2)
    idp = np.eye(128, dtype=np.float32) * 0.5
    idm = np.eye(128, dtype=np.float32) * -0.5

    def relayout(x):
        # [T, C, NPIX] -> [pair, p2*64+c, t, x448]
        x = np.asarray(x, np.float32).reshape(T, C, NPAIR, 2, BL)
        return np.ascontiguousarray(x.transpose(2, 3, 1, 0, 4)
                                    .reshape(NPAIR, 128, T, BL))
    par = np.zeros((128, 516), dtype=np.float32)
    par[:, 0:128] = wd
    par[:, 128:256] = we
    par[:, 256:384] = idp
    par[:, 384:512] = idm
    par[:, 512] = bgh
    par[:, 513] = -TH
    par[:, 514] = np.tile(np.asarray(gamma, np.float32), 2)
    par[:, 515] = np.tile(np.asarray(beta, np.float32), 2)
    in_maps = []
    for b in range(NCORES):
        in_maps.append({
            "dec": relayout(np.asarray(dec[:, b]).reshape(T, C, NPIX)),
            "enc": relayout(np.asarray(enc[:, b]).reshape(T, C, NPIX)),
            "par": par,
        })
    return in_maps


def kernel(dec, enc, Wg, bg, gamma, beta, _trace=False, _trace_kwargs=None):
    from concourse.bass_utils import run_bass_kernel_spmd

    if "nc" not in _cache:
        _cache["nc"] = _build()
    nc = _cache["nc"]

    in_maps = _prep_host(dec, enc, Wg, bg, gamma, beta)
    kw = {}
    if _trace:
        kw["trace"] = True
        if _trace_kwargs:
            kw.update(_trace_kwargs)
    res = run_bass_kernel_spmd(nc, in_maps, core_ids=list(range(NCORES)), **kw)
    outs = []
    for b in range(NCORES):
        o = np.asarray(res.results[b]["out"]).astype(np.float32)
        # [pair, p2*64+c, t, x448] -> [T, C, NPIX]
        o = o.reshape(NPAIR, 2, C, T, BL).transpose(3, 2, 0, 1, 4)
        outs.append(o.reshape(T, C, NPIX))
    out = np.stack(outs, axis=1).reshape(T, B, C, H, W)
    if _trace:
        _cache["last_res"] = res
    return out


# revision 58
# speedup vs baseline: 1.0153x; 1.0153x over previous
"""Trainium2 Bass kernel for GatedSkipFusion (gate conv -> sigmoid blend ->
4-step LIF -> BatchNorm with training stats).

Self-contained: hardcodes shapes T=4, B=8, C=64, H=W=112; shards batch B
across 8 NeuronCores; BN stats via a 64-float AllReduce.

Math:
  gate = sigmoid(pre); fused = enc + gate*(dec-enc). With h = tanh(pre/2):
  gate = 0.5 + 0.5*h, so fused = enc + 0.5*(1+h)*D, D = dec-enc.
  LIF (tau=2, hard reset, v_th=0.15): v_t = 0.5*v_{t-1}*m_{t-1} + fused_t,
  m = (v < th). Spikes are binary so BN var = mu - mu^2; the BN output is a
  per-channel affine of the sign record sg = Sign(v - th) in {-1,0,1}:
  out = (a/2)*sg + (a/2 + beta - mu*a), a = gamma*rsqrt(var+eps).

Engine split (software-pipelined with per-stage pair lags so no engine
stream head-of-line blocks; the per-core program is then bound by DVE
occupancy ~107us against a 90us DMA floor at 360 GB/s):
  PE    : gate matmuls and D'=0.5*(dec-enc) via +-0.5*I, all fp32r
          (1 cyc/row; exact-enough: identity matmuls add no spike flips)
  Act   : batched tanh from a 4-bank PSUM tile; per-step Sign with
          accumulation for the BN statistics (lagged one pipeline
          iteration so it never paces DVE)
  DVE   : gD=(1+h)*D', F=gD+enc, the LIF reset-mask and v-update stts,
          and the final affine as a 4x-mode fp16 tensor_scalar
  Pool  : second DMA-issue queue (SWDGE) for stats/pass-2 transfers
  fp16 output (halves the output DMA; ~6e-4 systematic error).
"""

import numpy as np

T, B, C, H, W = 4, 8, 64, 112, 112
NPIX = H * W          # 12544
BL = 448              # pixel block (free dim)
NPAIR = NPIX // (2 * BL)   # 14 pairs of blocks
NTILE = NPAIR * T     # 56 (pair,t) tiles
TH = 0.15
EPS = 1e-5
NCORES = 8
N_TOTAL = T * B * NPIX     # 401408 per-channel element count
N_CORE = T * NPIX          # 50176 per-core per-channel count

_cache = {}


def _build(reps=1, use_collective=True, num_devices=NCORES, d_on_pe=True,
           skip=frozenset()):
    import concourse.bass as bass
    import concourse.bacc as bacc
    import concourse.mybir as mybir
    import concourse.tile as tile

    F32 = mybir.dt.float32
    F32R = mybir.dt.float32r
    F16 = mybir.dt.float16
    AF = mybir.ActivationFunctionType
    OP = mybir.AluOpType
    AX = mybir.AxisListType

    nc = bacc.Bacc("TRN2", target_bir_lowering=False, debug=False,
                   enable_asserts=False, num_devices=num_devices)

    # host pre-arranged layout: [pair, partition(p2*64+c), t, x]
    dec_d = nc.dram_tensor("dec", [NPAIR, 128, T, BL], F32R,
                           kind="ExternalInput")
    enc_d = nc.dram_tensor("enc", [NPAIR, 128, T, BL], F32R,
                           kind="ExternalInput")
    # all parameters packed into one tensor: one DMA at startup
    # cols 0:128 wd | 128:256 we | 256:384 idp | 384:512 idm
    # | 512 bgh | 513 nth | 514 gam | 515 bet | 516:644 i2x
    par_d = nc.dram_tensor("par", [128, 644], F32R, kind="ExternalInput")
    out_d = nc.dram_tensor("out", [NPAIR, 128, T, BL], F16,
                           kind="ExternalOutput")

    with tile.TileContext(nc) as tc:
        with tc.tile_pool(name="const", bufs=1) as cp, \
             tc.tile_pool(name="io", bufs=3) as io, \
             tc.tile_pool(name="wk", bufs=3) as wk, \
             tc.tile_pool(name="wkg", bufs=2) as wkg, \
             tc.tile_pool(name="wkf", bufs=3) as wkf, \
             tc.tile_pool(name="vv", bufs=3) as vv, \
             tc.tile_pool(name="sm", bufs=6) as sm, \
             tc.tile_pool(name="ot", bufs=5) as op_, \
             tc.tile_pool(name="ps", bufs=1, space="PSUM") as ps, \
             tc.tile_pool(name="psd", bufs=1, space="PSUM") as psd, \
             tc.tile_pool(name="dram", bufs=2, space="DRAM") as dp:

            par_t = cp.tile([128, 644], F32R)
            nc.sync.dma_start(par_t[:], par_d[:, :])
            wd_t = par_t[:, 0:128]
            we_t = par_t[:, 128:256]
            idp_t = par_t[:, 256:384]
            idm_t = par_t[:, 384:512]
            bgh_t = par_t[:, 512:513].bitcast(F32)
            nth_t = par_t[:, 513:514].bitcast(F32)
            gam_t = par_t[:, 514:515].bitcast(F32)
            bet_t = par_t[:, 515:516].bitcast(F32)
            i2x_t = par_t[:, 516:644]

            store = cp.tile([128, NTILE, BL], F16)    # sign record
            scol = cp.tile([128, NTILE], F32)         # per-tile sign sums



            for _rep in range(reps):
                # ---------------- pass 1 (software-pipelined) ----------------
                # Stage lags per emission iteration k:
                #   dma(k); pre/h/D(k-1); gD/F(k-2); lif(k-3); signs(k-4/k-3)
                # Every instruction's inputs were produced in an earlier
                # iteration, so no engine stream ever head-of-line blocks.
                dec4s, enc4s = {}, {}
                h4s, gD4s, F4s, P4s = {}, {}, {}, {}
                vps = {}      # pair -> list of v tiles (per t)

                def emit_dma(p):
                    # halves so the first matmuls unblock after 1/4 of the
                    # pair's bytes (shortens pipeline fill)
                    dec4 = io.tile([128, T, BL], F32R)
                    enc4 = io.tile([128, T, BL], F32R)
                    if p == 0:
                        # dec first: the gate matmuls only need dec
                        nc.sync.dma_start(dec4[:, 0:2], dec_d[p, :, 0:2])
                        nc.sync.dma_start(dec4[:, 2:4], dec_d[p, :, 2:4])
                        nc.sync.dma_start(enc4[:, 0:2], enc_d[p, :, 0:2])
                        nc.sync.dma_start(enc4[:, 2:4], enc_d[p, :, 2:4])
                    else:
                        nc.sync.dma_start(dec4[:, 0:2], dec_d[p, :, 0:2])
                        nc.sync.dma_start(enc4[:, 0:2], enc_d[p, :, 0:2])
                        nc.sync.dma_start(dec4[:, 2:4], dec_d[p, :, 2:4])
                        nc.sync.dma_start(enc4[:, 2:4], enc_d[p, :, 2:4])
                    dec4s[p], enc4s[p] = dec4, enc4

                def emit_signs(p):
                    # sign for (p, 1..3) plus (p+1, 0): all deps one iter old
                    for t in range(1, T):
                        if 0 <= p < NPAIR:
                            idx = p * T + t
                            nc.scalar.activation(
                                store[:, idx], vps[p][t], AF.Sign,
                                bias=nth_t, scale=1.0,
                                accum_out=scol[:, idx:idx + 1])
                    q = p + 1
                    if 0 <= q < NPAIR:
                        idx = q * T
                        nc.scalar.activation(
                            store[:, idx], F4s[q][:, 0], AF.Sign,
                            bias=nth_t, scale=1.0,
                            accum_out=scol[:, idx:idx + 1])

                def emit_pre_h(p):
                    dec4, enc4 = dec4s[p], enc4s[p]
                    P4 = ps.tile([128, T, 512], F32)
                    if p == 0:
                        # warm the PE p-state on the param tile while the
                        # first input DMAs stream; the real matmuls below
                        # overwrite these banks (start=True resets PSUM)
                        for w in range(6):
                            nc.tensor.matmul(out=P4[:, w % T, 0:BL],
                                             lhsT=idp_t,
                                             rhs=par_t[:, 0:448],
                                             start=True, stop=True)
                    for t in range(T):
                        nc.tensor.matmul(out=P4[:, t, 0:BL], lhsT=wd_t,
                                         rhs=dec4[:, t], start=True,
                                         stop=False)
                        nc.tensor.matmul(out=P4[:, t, 0:BL], lhsT=we_t,
                                         rhs=enc4[:, t], start=False,
                                         stop=True)
                    h4 = wk.tile([128, T, BL], F32)
                    nc.scalar.activation(h4[:], P4[:, :, 0:BL], AF.Tanh,
                                         bias=bgh_t, scale=0.5)
                    h4s[p] = h4
                    P4s[p] = P4

                def emit_D(p):
                    dec4, enc4 = dec4s[p], enc4s[p]
                    if p < 2:
                        # fill phase: DVE is idle, and this keeps the PE +
                        # PSUM path off the critical startup chain
                        D4t = wk.tile([128, T, BL], F32)
                        nc.vector.tensor_tensor(D4t[:], dec4[:].bitcast(F32),
                                                enc4[:].bitcast(F32),
                                                OP.subtract)
                        return ("sbuf", D4t)
                    D4ap = psd.tile([128, T, 512], F32)
                    for t in range(T):
                        nc.tensor.matmul(out=D4ap[:, t, 0:BL],
                                         lhsT=idp_t, rhs=dec4[:, t],
                                         start=True, stop=False)
                        nc.tensor.matmul(out=D4ap[:, t, 0:BL],
                                         lhsT=idm_t, rhs=enc4[:, t],
                                         start=False, stop=True)
                    return ("psum", D4ap)

                def emit_gD_F(p, D4pack):
                    # gD = (1+h)*0.5*(dec-enc) = sigma(pre)*(dec-enc)
                    kind, D4ap = D4pack
                    if kind == "sbuf":
                        # D unscaled: fold the 0.5 into F's scalar instead
                        D4v, fscale = D4ap[:], 0.5
                    else:
                        D4v, fscale = D4ap[:, :, 0:BL], 1.0
                    gD4 = wkg.tile([128, T, BL], F32)
                    nc.vector.scalar_tensor_tensor(out=gD4[:], in0=h4s[p][:],
                                                   scalar=1.0,
                                                   in1=D4v,
                                                   op0=OP.add, op1=OP.mult)
                    F4 = wkf.tile([128, T, BL], F32)
                    nc.vector.scalar_tensor_tensor(out=F4[:], in0=gD4[:],
                                                   scalar=fscale,
                                                   in1=enc4s[p][:].bitcast(F32),
                                                   op0=OP.mult, op1=OP.add)
                    F4s[p] = F4
                    del h4s[p]
                    del dec4s[p], enc4s[p]

                def emit_lif(p):
                    F4 = F4s[p]
                    vp = F4[:, 0]
                    vlist = [vp]
                    for t in range(T - 1):
                        vrn = vv.tile([128, BL], F32)
                        nc.vector.scalar_tensor_tensor(
                            out=vrn[:], in0=vp, scalar=TH, in1=vp,
                            op0=OP.is_lt, op1=OP.mult)
                        vpt = sm.tile([128, BL], F32)
                        nc.vector.scalar_tensor_tensor(
                            out=vpt[:], in0=vrn[:], scalar=0.5,
                            in1=F4[:, t + 1], op0=OP.mult, op1=OP.add)
                        vp = vpt[:]
                        vlist.append(vp)
                    vps[p] = vlist

                D4prev = {}
                for k in range(NPAIR + 4):
                    if k >= 3:
                        emit_signs(k - 4)   # signs for pair k-4 (t>=1)
                                            # and pair k-3 (t=0)
                    if k < NPAIR:
                        emit_dma(k)
                    if 0 <= k - 1 < NPAIR:
                        emit_pre_h(k - 1)
                        D4prev[k - 1] = emit_D(k - 1)
                    if 0 <= k - 2 < NPAIR:
                        emit_gD_F(k - 2, D4prev.pop(k - 2))
                    if 0 <= k - 3 < NPAIR:
                        emit_lif(k - 3)

                # ---------------- stats ----------------
                # per-channel sign sum: contract the two 64-partition halves
                # of scol with a stacked identity on PE, then reduce tiles
                # per-channel sign sum duplicated on both partition
                # halves via one matmul with a [2,2]-tiled identity; lands
                # in the last pair's (long since consumed) P4 bank
                scolR = cp.tile([128, NTILE], F32R)
                nc.vector.tensor_scalar(out=scolR[:], in0=scol[:],
                                        scalar1=1.0, scalar2=None,
                                        op0=OP.mult)
                ssum = P4s[NPAIR - 1][:, 0, 0:NTILE]
                nc.tensor.matmul(out=ssum, lhsT=i2x_t,
                                 rhs=scolR[:],
                                 start=True, stop=True)
                s128 = cp.tile([128, 1], F32)
                nc.vector.tensor_reduce(out=s128[:], in_=ssum,
                                        axis=AX.X, op=OP.add)
                mu = cp.tile([128, 1], F32)
                if use_collective:
                    # local spike count = 0.5*sum_sign + N_CORE/2
                    loc = cp.tile([64, 1], F32)
                    nc.vector.tensor_scalar(out=loc[:], in0=s128[0:64, :],
                                            scalar1=0.5,
                                            scalar2=float(N_CORE) / 2.0,
                                            op0=OP.mult, op1=OP.add)
                    cin = dp.tile([64, 1], F32)
                    cout = dp.tile([64, 1], F32)
                    nc.sync.dma_start(cin[:], loc[:])
                    nc.gpsimd.collective_compute(
                        "AllReduce", OP.add,
                        replica_groups=[list(range(num_devices))],
                        ins=[cin.opt()], outs=[cout.opt()])
                    S128 = cp.tile([128, 1], F32)
                    nc.sync.dma_start(S128[0:64, :], cout[:])
                    nc.gpsimd.dma_start(S128[64:128, :], cout[:])
                    nc.vector.tensor_scalar(out=mu[:], in0=S128[:],
                                            scalar1=1.0 / float(N_TOTAL),
                                            scalar2=None, op0=OP.mult)
                else:
                    # mu = ((0.5*sum + N_CORE/2) * NCORES) / N_TOTAL
                    nc.vector.tensor_scalar(
                        out=mu[:], in0=s128[:],
                        scalar1=0.5 * NCORES / float(N_TOTAL),
                        scalar2=N_CORE * 0.5 * NCORES / float(N_TOTAL),
                        op0=OP.mult, op1=OP.add)
                # x = mu*(1-mu) + eps
                m1 = cp.tile([128, 1], F32)
                nc.vector.tensor_scalar(out=m1[:], in0=mu[:], scalar1=-1.0,
                                        scalar2=1.0, op0=OP.mult, op1=OP.add)
                x = cp.tile([128, 1], F32)
                nc.vector.tensor_tensor(x[:], m1[:], mu[:], OP.mult)
                nc.vector.tensor_scalar(out=x[:], in0=x[:], scalar1=EPS,
                                        scalar2=None, op0=OP.add)
                # r = 1/sqrt(x) + one Newton step r *= 1.5-0.5*x*r^2
                sq = cp.tile([128, 1], F32)
                nc.scalar.activation(sq[:], x[:], AF.Sqrt)
                r0 = cp.tile([128, 1], F32)
                nc.vector.reciprocal(r0[:], sq[:])
                e = cp.tile([128, 1], F32)
                nc.vector.tensor_tensor(e[:], r0[:], r0[:], OP.mult)
                nc.vector.tensor_tensor(e[:], e[:], x[:], OP.mult)
                nc.vector.tensor_scalar(out=e[:], in0=e[:], scalar1=-0.5,
                                        scalar2=1.5, op0=OP.mult, op1=OP.add)
                r = cp.tile([128, 1], F32)
                nc.vector.tensor_tensor(r[:], r0[:], e[:], OP.mult)
                # a = gamma*r ; scale = a/2 ; bias = a/2 + beta - mu*a
                a = cp.tile([128, 1], F32)
                nc.vector.tensor_tensor(a[:], gam_t, r[:], OP.mult)
                sc128 = cp.tile([128, 1], F32)
                nc.vector.tensor_scalar(out=sc128[:], in0=a[:], scalar1=0.5,
                                        scalar2=None, op0=OP.mult)
                tmp = cp.tile([128, 1], F32)
                nc.vector.tensor_tensor(tmp[:], mu[:], a[:], OP.mult)
                b0 = cp.tile([128, 1], F32)
                nc.vector.tensor_tensor(b0[:], bet_t, tmp[:], OP.subtract)
                bi128 = cp.tile([128, 1], F32)
                nc.vector.tensor_tensor(bi128[:], sc128[:], b0[:], OP.add)

                # ---------------- pass 2 ----------------
                for pair in range(NPAIR):
                    ot = op_.tile([128, T, BL], F16)
                    nc.vector.tensor_scalar(
                        out=ot[:], in0=store[:, pair * T:(pair + 1) * T, :],
                        scalar1=sc128[:], scalar2=bi128[:],
                        op0=OP.mult, op1=OP.add)
                    eng = (nc.sync, nc.gpsimd, nc.scalar)[pair % 3]
                    eng.dma_start(out_d[pair], ot[:])

    nc.compile()
    return nc


def _prep_host(dec, enc, Wg, bg, gamma, beta):
    Wg = np.asarray(Wg, dtype=np.float32)
    wdT = np.ascontiguousarray(Wg[:, :64].T)   # [k, m] dec-part
    weT = np.ascontiguousarray(Wg[:, 64:].T)   # enc-part
    wd = np.zeros((128, 128), dtype=np.float32)
    we = np.zeros((128, 128), dtype=np.float32)
    wd[:64, :64] = wdT
    wd[64:, 64:] = wdT
    we[:64, :64] = weT
    we[64:, 64:] = weT
    bgh = np.tile(0.5 * np.asarray(bg, np.float32), 2)
    idp = np.eye(128, dtype=np.float32) * 0.5
    idm = np.eye(128, dtype=np.float32) * -0.5

    def relayout(x):
        # [T, C, NPIX] -> [pair, p2*64+c, t, x448]
        x = np.asarray(x, np.float32).reshape(T, C, NPAIR, 2, BL)
        return np.ascontiguousarray(x.transpose(2, 3, 1, 0, 4)
                                    .reshape(NPAIR, 128, T, BL))
    par = np.zeros((128, 644), dtype=np.float32)
    par[:, 0:128] = wd
    par[:, 128:256] = we
    par[:, 256:384] = idp
    par[:, 384:512] = idm
    par[:, 512] = bgh
    par[:, 513] = -TH
    par[:, 514] = np.tile(np.asarray(gamma, np.float32), 2)
    par[:, 515] = np.tile(np.asarray(beta, np.float32), 2)
    par[:, 516:644] = np.tile(np.eye(64, dtype=np.float32), (2, 2))
    in_maps = []
    for b in range(NCORES):
        in_maps.append({
            "dec": relayout(np.asarray(dec[:, b]).reshape(T, C, NPIX)),
            "enc": relayout(np.asarray(enc[:, b]).reshape(T, C, NPIX)),
            "par": par,
        })
    return in_maps


def kernel(dec, enc, Wg, bg, gamma, beta, _trace=False, _trace_kwargs=None):
    from concourse.bass_utils import run_bass_kernel_spmd

    if "nc" not in _cache:
        _cache["nc"] = _build()
    nc = _cache["nc"]

    in_maps = _prep_host(dec, enc, Wg, bg, gamma, beta)
    kw = {}
    if _trace:
        kw["trace"] = True
        if _trace_kwargs:
            kw.update(_trace_kwargs)
    res = run_bass_kernel_spmd(nc, in_maps, core_ids=list(range(NCORES)), **kw)
    outs = []
    for b in range(NCORES):
        o = np.asarray(res.results[b]["out"]).astype(np.float32)
        # [pair, p2*64+c, t, x448] -> [T, C, NPIX]
        o = o.reshape(NPAIR, 2, C, T, BL).transpose(3, 2, 0, 1, 4)
        outs.append(o.reshape(T, C, NPIX))
    out = np.stack(outs, axis=1).reshape(T, B, C, H, W)
    if _trace:
        _cache["last_res"] = res
    return out


# revision 59
# speedup vs baseline: 1.0194x; 1.0040x over previous
"""Trainium2 Bass kernel for GatedSkipFusion (gate conv -> sigmoid blend ->
4-step LIF -> BatchNorm with training stats).

Self-contained: hardcodes shapes T=4, B=8, C=64, H=W=112; shards batch B
across 8 NeuronCores; BN stats via a 64-float AllReduce.

Math:
  gate = sigmoid(pre); fused = enc + gate*(dec-enc). With h = tanh(pre/2):
  gate = 0.5 + 0.5*h, so fused = enc + 0.5*(1+h)*D, D = dec-enc.
  LIF (tau=2, hard reset, v_th=0.15): v_t = 0.5*v_{t-1}*m_{t-1} + fused_t,
  m = (v < th). Spikes are binary so BN var = mu - mu^2; the BN output is a
  per-channel affine of the sign record sg = Sign(v - th) in {-1,0,1}:
  out = (a/2)*sg + (a/2 + beta - mu*a), a = gamma*rsqrt(var+eps).

Engine split (software-pipelined with per-stage pair lags so no engine
stream head-of-line blocks; the per-core program is then bound by DVE
occupancy ~107us against a 90us DMA floor at 360 GB/s):
  PE    : gate matmuls and D'=0.5*(dec-enc) via +-0.5*I, all fp32r
          (1 cyc/row; exact-enough: identity matmuls add no spike flips)
  Act   : batched tanh from a 4-bank PSUM tile; per-step Sign with
          accumulation for the BN statistics (lagged one pipeline
          iteration so it never paces DVE)
  DVE   : gD=(1+h)*D', F=gD+enc, the LIF reset-mask and v-update stts,
          and the final affine as a 4x-mode fp16 tensor_scalar
  Pool  : second DMA-issue queue (SWDGE) for stats/pass-2 transfers
  fp16 output (halves the output DMA; ~6e-4 systematic error).
"""

import numpy as np

T, B, C, H, W = 4, 8, 64, 112, 112
NPIX = H * W          # 12544
BL = 448              # pixel block (free dim)
NPAIR = NPIX // (2 * BL)   # 14 pairs of blocks
NTILE = NPAIR * T     # 56 (pair,t) tiles
TH = 0.15
EPS = 1e-5
NCORES = 8
N_TOTAL = T * B * NPIX     # 401408 per-channel element count
N_CORE = T * NPIX          # 50176 per-core per-channel count

_cache = {}


def _build(reps=1, use_collective=True, num_devices=NCORES, d_on_pe=True,
           skip=frozenset()):
    import concourse.bass as bass
    import concourse.bacc as bacc
    import concourse.mybir as mybir
    import concourse.tile as tile

    F32 = mybir.dt.float32
    F32R = mybir.dt.float32r
    F16 = mybir.dt.float16
    AF = mybir.ActivationFunctionType
    OP = mybir.AluOpType
    AX = mybir.AxisListType

    nc = bacc.Bacc("TRN2", target_bir_lowering=False, debug=False,
                   enable_asserts=False, num_devices=num_devices)

    # host pre-arranged layout: [pair, partition(p2*64+c), t, x]
    dec_d = nc.dram_tensor("dec", [NPAIR, 128, T, BL], F32R,
                           kind="ExternalInput")
    enc_d = nc.dram_tensor("enc", [NPAIR, 128, T, BL], F32R,
                           kind="ExternalInput")
    # all parameters packed into one tensor: one DMA at startup
    # cols 0:128 wd | 128:256 we | 256:384 idp | 384:512 idm
    # | 512 bgh | 513 nth | 514 gam | 515 bet | 516:644 i2x
    par_d = nc.dram_tensor("par", [128, 644], F32R, kind="ExternalInput")
    out_d = nc.dram_tensor("out", [NPAIR, 128, T, BL], F16,
                           kind="ExternalOutput")

    with tile.TileContext(nc) as tc:
        with tc.tile_pool(name="const", bufs=1) as cp, \
             tc.tile_pool(name="io", bufs=3) as io, \
             tc.tile_pool(name="wk", bufs=3) as wk, \
             tc.tile_pool(name="wkg", bufs=2) as wkg, \
             tc.tile_pool(name="wkf", bufs=3) as wkf, \
             tc.tile_pool(name="vv", bufs=3) as vv, \
             tc.tile_pool(name="sm", bufs=6) as sm, \
             tc.tile_pool(name="ot", bufs=5) as op_, \
             tc.tile_pool(name="ps", bufs=1, space="PSUM") as ps, \
             tc.tile_pool(name="psd", bufs=1, space="PSUM") as psd, \
             tc.tile_pool(name="dram", bufs=2, space="DRAM") as dp:

            par_t = cp.tile([128, 644], F32R)
            nc.sync.dma_start(par_t[:], par_d[:, :])
            wd_t = par_t[:, 0:128]
            we_t = par_t[:, 128:256]
            idp_t = par_t[:, 256:384]
            idm_t = par_t[:, 384:512]
            bgh_t = par_t[:, 512:513].bitcast(F32)
            nth_t = par_t[:, 513:514].bitcast(F32)
            gam_t = par_t[:, 514:515].bitcast(F32)
            bet_t = par_t[:, 515:516].bitcast(F32)
            i2x_t = par_t[:, 516:644]

            store = cp.tile([128, NTILE, BL], F16)    # sign record
            scol = cp.tile([128, NTILE], F32)         # per-tile sign sums



            for _rep in range(reps):
                # ---------------- pass 1 (software-pipelined) ----------------
                # Stage lags per emission iteration k:
                #   dma(k); pre/h/D(k-1); gD/F(k-2); lif(k-3); signs(k-4/k-3)
                # Every instruction's inputs were produced in an earlier
                # iteration, so no engine stream ever head-of-line blocks.
                dec4s, enc4s = {}, {}
                h4s, gD4s, F4s, P4s = {}, {}, {}, {}
                vps = {}      # pair -> list of v tiles (per t)

                def emit_dma(p):
                    # halves so the first matmuls unblock after 1/4 of the
                    # pair's bytes (shortens pipeline fill)
                    dec4 = io.tile([128, T, BL], F32R)
                    enc4 = io.tile([128, T, BL], F32R)
                    if p == 0:
                        # dec first: the gate matmuls only need dec
                        nc.sync.dma_start(dec4[:, 0:2], dec_d[p, :, 0:2])
                        nc.sync.dma_start(dec4[:, 2:4], dec_d[p, :, 2:4])
                        nc.sync.dma_start(enc4[:, 0:2], enc_d[p, :, 0:2])
                        nc.sync.dma_start(enc4[:, 2:4], enc_d[p, :, 2:4])
                    else:
                        nc.sync.dma_start(dec4[:, 0:2], dec_d[p, :, 0:2])
                        nc.sync.dma_start(enc4[:, 0:2], enc_d[p, :, 0:2])
                        nc.sync.dma_start(dec4[:, 2:4], dec_d[p, :, 2:4])
                        nc.sync.dma_start(enc4[:, 2:4], enc_d[p, :, 2:4])
                    dec4s[p], enc4s[p] = dec4, enc4

                def emit_signs(p):
                    # sign for (p, 1..3) plus (p+1, 0): all deps one iter old
                    for t in range(1, T):
                        if 0 <= p < NPAIR:
                            idx = p * T + t
                            nc.scalar.activation(
                                store[:, idx], vps[p][t], AF.Sign,
                                bias=nth_t, scale=1.0,
                                accum_out=scol[:, idx:idx + 1])
                    q = p + 1
                    if 0 <= q < NPAIR:
                        idx = q * T
                        nc.scalar.activation(
                            store[:, idx], F4s[q][:, 0], AF.Sign,
                            bias=nth_t, scale=1.0,
                            accum_out=scol[:, idx:idx + 1])

                def emit_pre_h(p):
                    dec4, enc4 = dec4s[p], enc4s[p]
                    P4 = ps.tile([128, T, 512], F32)
                    if p == 0:
                        # warm the PE p-state on the param tile while the
                        # first input DMAs stream; the real matmuls below
                        # overwrite these banks (start=True resets PSUM)
                        for w in range(6):
                            nc.tensor.matmul(out=P4[:, w % T, 0:BL],
                                             lhsT=idp_t,
                                             rhs=par_t[:, 0:448],
                                             start=True, stop=True)
                    for t in range(T):
                        nc.tensor.matmul(out=P4[:, t, 0:BL], lhsT=wd_t,
                                         rhs=dec4[:, t], start=True,
                                         stop=False)
                        nc.tensor.matmul(out=P4[:, t, 0:BL], lhsT=we_t,
                                         rhs=enc4[:, t], start=False,
                                         stop=True)
                    h4 = wk.tile([128, T, BL], F32)
                    if p == 0:
                        # halves so the fill-critical chain starts as soon
                        # as the first half of the pair's matmuls finish
                        nc.scalar.activation(h4[:, 0:2], P4[:, 0:2, 0:BL],
                                             AF.Tanh, bias=bgh_t, scale=0.5)
                        nc.scalar.activation(h4[:, 2:4], P4[:, 2:4, 0:BL],
                                             AF.Tanh, bias=bgh_t, scale=0.5)
                    else:
                        nc.scalar.activation(h4[:], P4[:, :, 0:BL], AF.Tanh,
                                             bias=bgh_t, scale=0.5)
                    h4s[p] = h4
                    P4s[p] = P4

                def emit_D(p):
                    dec4, enc4 = dec4s[p], enc4s[p]
                    if p < 2:
                        # fill phase: DVE is idle, and this keeps the PE +
                        # PSUM path off the critical startup chain
                        D4t = wk.tile([128, T, BL], F32)
                        if p == 0:
                            nc.vector.tensor_tensor(
                                D4t[:, 0:2], dec4[:, 0:2].bitcast(F32),
                                enc4[:, 0:2].bitcast(F32), OP.subtract)
                            nc.vector.tensor_tensor(
                                D4t[:, 2:4], dec4[:, 2:4].bitcast(F32),
                                enc4[:, 2:4].bitcast(F32), OP.subtract)
                        else:
                            nc.vector.tensor_tensor(D4t[:],
                                                    dec4[:].bitcast(F32),
                                                    enc4[:].bitcast(F32),
                                                    OP.subtract)
                        return ("sbuf", D4t)
                    D4ap = psd.tile([128, T, 512], F32)
                    for t in range(T):
                        nc.tensor.matmul(out=D4ap[:, t, 0:BL],
                                         lhsT=idp_t, rhs=dec4[:, t],
                                         start=True, stop=False)
                        nc.tensor.matmul(out=D4ap[:, t, 0:BL],
                                         lhsT=idm_t, rhs=enc4[:, t],
                                         start=False, stop=True)
                    return ("psum", D4ap)

                def emit_gD_F(p, D4pack):
                    # gD = (1+h)*0.5*(dec-enc) = sigma(pre)*(dec-enc)
                    kind, D4ap = D4pack
                    if kind == "sbuf":
                        # D unscaled: fold the 0.5 into F's scalar instead
                        D4v, fscale = D4ap[:], 0.5
                    else:
                        D4v, fscale = D4ap[:, :, 0:BL], 1.0
                    gD4 = wkg.tile([128, T, BL], F32)
                    F4 = wkf.tile([128, T, BL], F32)
                    if p == 0:
                        for sl in (slice(0, 2), slice(2, 4)):
                            nc.vector.scalar_tensor_tensor(
                                out=gD4[:, sl], in0=h4s[p][:, sl],
                                scalar=1.0, in1=D4ap[:, sl],
                                op0=OP.add, op1=OP.mult)
                            nc.vector.scalar_tensor_tensor(
                                out=F4[:, sl], in0=gD4[:, sl],
                                scalar=fscale,
                                in1=enc4s[p][:, sl].bitcast(F32),
                                op0=OP.mult, op1=OP.add)
                    else:
                        nc.vector.scalar_tensor_tensor(out=gD4[:],
                                                       in0=h4s[p][:],
                                                       scalar=1.0, in1=D4v,
                                                       op0=OP.add,
                                                       op1=OP.mult)
                        nc.vector.scalar_tensor_tensor(
                            out=F4[:], in0=gD4[:], scalar=fscale,
                            in1=enc4s[p][:].bitcast(F32),
                            op0=OP.mult, op1=OP.add)
                    F4s[p] = F4
                    del h4s[p]
                    del dec4s[p], enc4s[p]

                def emit_lif(p):
                    F4 = F4s[p]
                    vp = F4[:, 0]
                    vlist = [vp]
                    for t in range(T - 1):
                        vrn = vv.tile([128, BL], F32)
                        nc.vector.scalar_tensor_tensor(
                            out=vrn[:], in0=vp, scalar=TH, in1=vp,
                            op0=OP.is_lt, op1=OP.mult)
                        vpt = sm.tile([128, BL], F32)
                        nc.vector.scalar_tensor_tensor(
                            out=vpt[:], in0=vrn[:], scalar=0.5,
                            in1=F4[:, t + 1], op0=OP.mult, op1=OP.add)
                        vp = vpt[:]
                        vlist.append(vp)
                    vps[p] = vlist

                D4prev = {}
                for k in range(NPAIR + 4):
                    if k >= 3:
                        emit_signs(k - 4)   # signs for pair k-4 (t>=1)
                                            # and pair k-3 (t=0)
                    if k < NPAIR:
                        emit_dma(k)
                    if 0 <= k - 1 < NPAIR:
                        emit_pre_h(k - 1)
                        D4prev[k - 1] = emit_D(k - 1)
                    if 0 <= k - 2 < NPAIR:
                        emit_gD_F(k - 2, D4prev.pop(k - 2))
                    if 0 <= k - 3 < NPAIR:
                        emit_lif(k - 3)

                # ---------------- stats ----------------
                # per-channel sign sum: contract the two 64-partition halves
                # of scol with a stacked identity on PE, then reduce tiles
                # per-channel sign sum duplicated on both partition
                # halves via one matmul with a [2,2]-tiled identity; lands
                # in the last pair's (long since consumed) P4 bank
                scolR = cp.tile([128, NTILE], F32R)
                nc.vector.tensor_scalar(out=scolR[:], in0=scol[:],
                                        scalar1=1.0, scalar2=None,
                                        op0=OP.mult)
                ssum = P4s[NPAIR - 1][:, 0, 0:NTILE]
                nc.tensor.matmul(out=ssum, lhsT=i2x_t,
                                 rhs=scolR[:],
                                 start=True, stop=True)
                s128 = cp.tile([128, 1], F32)
                nc.vector.tensor_reduce(out=s128[:], in_=ssum,
                                        axis=AX.X, op=OP.add)
                mu = cp.tile([128, 1], F32)
                if use_collective:
                    # local spike count = 0.5*sum_sign + N_CORE/2
                    loc = cp.tile([64, 1], F32)
                    nc.vector.tensor_scalar(out=loc[:], in0=s128[0:64, :],
                                            scalar1=0.5,
                                            scalar2=float(N_CORE) / 2.0,
                                            op0=OP.mult, op1=OP.add)
                    cin = dp.tile([64, 1], F32)
                    cout = dp.tile([64, 1], F32)
                    nc.sync.dma_start(cin[:], loc[:])
                    nc.gpsimd.collective_compute(
                        "AllReduce", OP.add,
                        replica_groups=[list(range(num_devices))],
                        ins=[cin.opt()], outs=[cout.opt()])
                    S128 = cp.tile([128, 1], F32)
                    nc.sync.dma_start(S128[0:64, :], cout[:])
                    nc.gpsimd.dma_start(S128[64:128, :], cout[:])
                    nc.vector.tensor_scalar(out=mu[:], in0=S128[:],
                                            scalar1=1.0 / float(N_TOTAL),
                                            scalar2=None, op0=OP.mult)
                else:
                    # mu = ((0.5*sum + N_CORE/2) * NCORES) / N_TOTAL
                    nc.vector.tensor_scalar(
                        out=mu[:], in0=s128[:],
                        scalar1=0.5 * NCORES / float(N_TOTAL),
                        scalar2=N_CORE * 0.5 * NCORES / float(N_TOTAL),
                        op0=OP.mult, op1=OP.add)
                # x = mu*(1-mu) + eps
                m1 = cp.tile([128, 1], F32)
                nc.vector.tensor_scalar(out=m1[:], in0=mu[:], scalar1=-1.0,
                                        scalar2=1.0, op0=OP.mult, op1=OP.add)
                x = cp.tile([128, 1], F32)
                nc.vector.tensor_tensor(x[:], m1[:], mu[:], OP.mult)
                nc.vector.tensor_scalar(out=x[:], in0=x[:], scalar1=EPS,
                                        scalar2=None, op0=OP.add)
                # r = 1/sqrt(x) + one Newton step r *= 1.5-0.5*x*r^2
                sq = cp.tile([128, 1], F32)
                nc.scalar.activation(sq[:], x[:], AF.Sqrt)
                r0 = cp.tile([128, 1], F32)
                nc.vector.reciprocal(r0[:], sq[:])
                e = cp.tile([128, 1], F32)
                nc.vector.tensor_tensor(e[:], r0[:], r0[:], OP.mult)
                nc.vector.tensor_tensor(e[:], e[:], x[:], OP.mult)
                nc.vector.tensor_scalar(out=e[:], in0=e[:], scalar1=-0.5,
                                        scalar2=1.5, op0=OP.mult, op1=OP.add)
                r = cp.tile([128, 1], F32)
                nc.vector.tensor_tensor(r[:], r0[:], e[:], OP.mult)
                # a = gamma*r ; scale = a/2 ; bias = a/2 + beta - mu*a
                a = cp.tile([128, 1], F32)
                nc.vector.tensor_tensor(a[:], gam_t, r[:], OP.mult)
                sc128 = cp.tile([128, 1], F32)
                nc.vector.tensor_scalar(out=sc128[:], in0=a[:], scalar1=0.5,
                                        scalar2=None, op0=OP.mult)
                tmp = cp.tile([128, 1], F32)
                nc.vector.tensor_tensor(tmp[:], mu[:], a[:], OP.mult)
                b0 = cp.tile([128, 1], F32)
                nc.vector.tensor_tensor(b0[:], bet_t, tmp[:], OP.subtract)
                bi128 = cp.tile([128, 1], F32)
                nc.vector.tensor_tensor(bi128[:], sc128[:], b0[:], OP.add)

                # ---------------- pass 2 ----------------
                for pair in range(NPAIR):
                    ot = op_.tile([128, T, BL], F16)
                    nc.vector.tensor_scalar(
                        out=ot[:], in0=store[:, pair * T:(pair + 1) * T, :],
                        scalar1=sc128[:], scalar2=bi128[:],
                        op0=OP.mult, op1=OP.add)
                    eng = (nc.sync, nc.gpsimd, nc.scalar)[pair % 3]
                    eng.dma_start(out_d[pair], ot[:])

    nc.compile()
    return nc


def _prep_host(dec, enc, Wg, bg, gamma, beta):
    Wg = np.asarray(Wg, dtype=np.float32)
    wdT = np.ascontiguousarray(Wg[:, :64].T)   # [k, m] dec-part
    weT = np.ascontiguousarray(Wg[:, 64:].T)   # enc-part
    wd = np.zeros((128, 128), dtype=np.float32)
    we = np.zeros((128, 128), dtype=np.float32)
    wd[:64, :64] = wdT
    wd[64:, 64:] = wdT
    we[:64, :64] = weT
    we[64:, 64:] = weT
    bgh = np.tile(0.5 * np.asarray(bg, np.float32), 2)
    idp = np.eye(128, dtype=np.float32) * 0.5
    idm = np.eye(128, dtype=np.float32) * -0.5

    def relayout(x):
        # [T, C, NPIX] -> [pair, p2*64+c, t, x448]
        x = np.asarray(x, np.float32).reshape(T, C, NPAIR, 2, BL)
        return np.ascontiguousarray(x.transpose(2, 3, 1, 0, 4)
                                    .reshape(NPAIR, 128, T, BL))
    par = np.zeros((128, 644), dtype=np.float32)
    par[:, 0:128] = wd
    par[:, 128:256] = we
    par[:, 256:384] = idp
    par[:, 384:512] = idm
    par[:, 512] = bgh
    par[:, 513] = -TH
    par[:, 514] = np.tile(np.asarray(gamma, np.float32), 2)
    par[:, 515] = np.tile(np.asarray(beta, np.float32), 2)
    par[:, 516:644] = np.tile(np.eye(64, dtype=np.float32), (2, 2))
    in_maps = []
    for b in range(NCORES):
        in_maps.append({
            "dec": relayout(np.asarray(dec[:, b]).reshape(T, C, NPIX)),
            "enc": relayout(np.asarray(enc[:, b]).reshape(T, C, NPIX)),
            "par": par,
        })
    return in_maps


def kernel(dec, enc, Wg, bg, gamma, beta, _trace=False, _trace_kwargs=None):
    from concourse.bass_utils import run_bass_kernel_spmd

    if "nc" not in _cache:
        _cache["nc"] = _build()
    nc = _cache["nc"]

    in_maps = _prep_host(dec, enc, Wg, bg, gamma, beta)
    kw = {}
    if _trace:
        kw["trace"] = True
        if _trace_kwargs:
            kw.update(_trace_kwargs)
    res = run_bass_kernel_spmd(nc, in_maps, core_ids=list(range(NCORES)), **kw)
    outs = []
    for b in range(NCORES):
        o = np.asarray(res.results[b]["out"]).astype(np.float32)
        # [pair, p2*64+c, t, x448] -> [T, C, NPIX]
        o = o.reshape(NPAIR, 2, C, T, BL).transpose(3, 2, 0, 1, 4)
        outs.append(o.reshape(T, C, NPIX))
    out = np.stack(outs, axis=1).reshape(T, B, C, H, W)
    if _trace:
        _cache["last_res"] = res
    return out


# revision 60
# speedup vs baseline: 1.0217x; 1.0023x over previous
"""Trainium2 Bass kernel for GatedSkipFusion (gate conv -> sigmoid blend ->
4-step LIF -> BatchNorm with training stats).

Self-contained: hardcodes shapes T=4, B=8, C=64, H=W=112; shards batch B
across 8 NeuronCores; BN stats via a 64-float AllReduce.

Math:
  gate = sigmoid(pre); fused = enc + gate*(dec-enc). With h = tanh(pre/2):
  gate = 0.5 + 0.5*h, so fused = enc + 0.5*(1+h)*D, D = dec-enc.
  LIF (tau=2, hard reset, v_th=0.15): v_t = 0.5*v_{t-1}*m_{t-1} + fused_t,
  m = (v < th). Spikes are binary so BN var = mu - mu^2; the BN output is a
  per-channel affine of the sign record sg = Sign(v - th) in {-1,0,1}:
  out = (a/2)*sg + (a/2 + beta - mu*a), a = gamma*rsqrt(var+eps).

Engine split (software-pipelined with per-stage pair lags so no engine
stream head-of-line blocks; the per-core program is then bound by DVE
occupancy ~107us against a 90us DMA floor at 360 GB/s):
  PE    : gate matmuls and D'=0.5*(dec-enc) via +-0.5*I, all fp32r
          (1 cyc/row; exact-enough: identity matmuls add no spike flips)
  Act   : batched tanh from a 4-bank PSUM tile; per-step Sign with
          accumulation for the BN statistics (lagged one pipeline
          iteration so it never paces DVE)
  DVE   : gD=(1+h)*D', F=gD+enc, the LIF reset-mask and v-update stts,
          and the final affine as a 4x-mode fp16 tensor_scalar
  Pool  : second DMA-issue queue (SWDGE) for stats/pass-2 transfers
  fp16 output (halves the output DMA; ~6e-4 systematic error).
"""

import numpy as np

T, B, C, H, W = 4, 8, 64, 112, 112
NPIX = H * W          # 12544
BL = 448              # pixel block (free dim)
NPAIR = NPIX // (2 * BL)   # 14 pairs of blocks
NTILE = NPAIR * T     # 56 (pair,t) tiles
TH = 0.15
EPS = 1e-5
NCORES = 8
N_TOTAL = T * B * NPIX     # 401408 per-channel element count
N_CORE = T * NPIX          # 50176 per-core per-channel count

_cache = {}


def _build(reps=1, use_collective=True, num_devices=NCORES, d_on_pe=True,
           skip=frozenset()):
    import concourse.bass as bass
    import concourse.bacc as bacc
    import concourse.mybir as mybir
    import concourse.tile as tile

    F32 = mybir.dt.float32
    F32R = mybir.dt.float32r
    F16 = mybir.dt.float16
    AF = mybir.ActivationFunctionType
    OP = mybir.AluOpType
    AX = mybir.AxisListType

    nc = bacc.Bacc("TRN2", target_bir_lowering=False, debug=False,
                   enable_asserts=False, num_devices=num_devices)

    # host pre-arranged layout: [pair, partition(p2*64+c), t, x]
    dec_d = nc.dram_tensor("dec", [NPAIR, 128, T, BL], F32R,
                           kind="ExternalInput")
    enc_d = nc.dram_tensor("enc", [NPAIR, 128, T, BL], F32R,
                           kind="ExternalInput")
    # all parameters packed into one tensor: one DMA at startup
    # cols 0:128 wd | 128:256 we | 256:384 idp | 384:512 idm
    # | 512 bgh | 513 nth | 514 gam | 515 bet | 516:644 i2x
    par_d = nc.dram_tensor("par", [128, 644], F32R, kind="ExternalInput")
    out_d = nc.dram_tensor("out", [NPAIR, 128, T, BL], F16,
                           kind="ExternalOutput")

    with tile.TileContext(nc) as tc:
        with tc.tile_pool(name="const", bufs=1) as cp, \
             tc.tile_pool(name="io", bufs=3) as io, \
             tc.tile_pool(name="wk", bufs=3) as wk, \
             tc.tile_pool(name="wkg", bufs=2) as wkg, \
             tc.tile_pool(name="wkf", bufs=3) as wkf, \
             tc.tile_pool(name="vv", bufs=3) as vv, \
             tc.tile_pool(name="sm", bufs=6) as sm, \
             tc.tile_pool(name="ot", bufs=5) as op_, \
             tc.tile_pool(name="ps", bufs=1, space="PSUM") as ps, \
             tc.tile_pool(name="psd", bufs=1, space="PSUM") as psd, \
             tc.tile_pool(name="dram", bufs=2, space="DRAM") as dp:

            par_t = cp.tile([128, 644], F32R)
            nc.sync.dma_start(par_t[:], par_d[:, :])
            wd_t = par_t[:, 0:128]
            we_t = par_t[:, 128:256]
            idp_t = par_t[:, 256:384]
            idm_t = par_t[:, 384:512]
            bgh_t = par_t[:, 512:513].bitcast(F32)
            nth_t = par_t[:, 513:514].bitcast(F32)
            gam_t = par_t[:, 514:515].bitcast(F32)
            bet_t = par_t[:, 515:516].bitcast(F32)
            i2x_t = par_t[:, 516:644]

            store = cp.tile([128, NTILE, BL], F16)    # sign record
            scol = cp.tile([128, NTILE], F32)         # per-tile sign sums



            for _rep in range(reps):
                # ---------------- pass 1 (software-pipelined) ----------------
                # Stage lags per emission iteration k:
                #   dma(k); pre/h/D(k-1); gD/F(k-2); lif(k-3); signs(k-4/k-3)
                # Every instruction's inputs were produced in an earlier
                # iteration, so no engine stream ever head-of-line blocks.
                dec4s, enc4s = {}, {}
                h4s, gD4s, F4s, P4s = {}, {}, {}, {}
                vps = {}      # pair -> list of v tiles (per t)

                def emit_dma(p):
                    # halves so the first matmuls unblock after 1/4 of the
                    # pair's bytes (shortens pipeline fill)
                    dec4 = io.tile([128, T, BL], F32R)
                    enc4 = io.tile([128, T, BL], F32R)
                    if p == 0:
                        # dec first: the gate matmuls only need dec
                        nc.sync.dma_start(dec4[:, 0:2], dec_d[p, :, 0:2])
                        nc.sync.dma_start(dec4[:, 2:4], dec_d[p, :, 2:4])
                        nc.sync.dma_start(enc4[:, 0:2], enc_d[p, :, 0:2])
                        nc.sync.dma_start(enc4[:, 2:4], enc_d[p, :, 2:4])
                    else:
                        nc.sync.dma_start(dec4[:, 0:2], dec_d[p, :, 0:2])
                        nc.sync.dma_start(enc4[:, 0:2], enc_d[p, :, 0:2])
                        nc.sync.dma_start(dec4[:, 2:4], dec_d[p, :, 2:4])
                        nc.sync.dma_start(enc4[:, 2:4], enc_d[p, :, 2:4])
                    dec4s[p], enc4s[p] = dec4, enc4

                def emit_signs(p):
                    # sign for (p, 1..3) plus (p+1, 0): all deps one iter old
                    for t in range(1, T):
                        if 0 <= p < NPAIR:
                            idx = p * T + t
                            nc.scalar.activation(
                                store[:, idx], vps[p][t], AF.Sign,
                                bias=nth_t, scale=1.0,
                                accum_out=scol[:, idx:idx + 1])
                    q = p + 1
                    if 0 <= q < NPAIR:
                        idx = q * T
                        nc.scalar.activation(
                            store[:, idx], F4s[q][:, 0], AF.Sign,
                            bias=nth_t, scale=1.0,
                            accum_out=scol[:, idx:idx + 1])

                def emit_pre_h(p):
                    dec4, enc4 = dec4s[p], enc4s[p]
                    P4 = ps.tile([128, T, 512], F32)
                    if p == 0:
                        # warm the PE p-state on the param tile while the
                        # first input DMAs stream; the real matmuls below
                        # overwrite these banks (start=True resets PSUM)
                        for w in range(3):
                            nc.tensor.matmul(out=P4[:, w % T, 0:BL],
                                             lhsT=idp_t,
                                             rhs=par_t[:, 0:448],
                                             start=True, stop=True)
                    for t in range(T):
                        nc.tensor.matmul(out=P4[:, t, 0:BL], lhsT=wd_t,
                                         rhs=dec4[:, t], start=True,
                                         stop=False)
                        nc.tensor.matmul(out=P4[:, t, 0:BL], lhsT=we_t,
                                         rhs=enc4[:, t], start=False,
                                         stop=True)
                    h4 = wk.tile([128, T, BL], F32)
                    if p == 0:
                        # halves so the fill-critical chain starts as soon
                        # as the first half of the pair's matmuls finish
                        nc.scalar.activation(h4[:, 0:2], P4[:, 0:2, 0:BL],
                                             AF.Tanh, bias=bgh_t, scale=0.5)
                        nc.scalar.activation(h4[:, 2:4], P4[:, 2:4, 0:BL],
                                             AF.Tanh, bias=bgh_t, scale=0.5)
                    else:
                        nc.scalar.activation(h4[:], P4[:, :, 0:BL], AF.Tanh,
                                             bias=bgh_t, scale=0.5)
                    h4s[p] = h4
                    P4s[p] = P4

                def emit_D(p):
                    dec4, enc4 = dec4s[p], enc4s[p]
                    if p < 2:
                        # fill phase: DVE is idle, and this keeps the PE +
                        # PSUM path off the critical startup chain
                        D4t = wk.tile([128, T, BL], F32)
                        if p == 0:
                            nc.vector.tensor_tensor(
                                D4t[:, 0:2], dec4[:, 0:2].bitcast(F32),
                                enc4[:, 0:2].bitcast(F32), OP.subtract)
                            nc.vector.tensor_tensor(
                                D4t[:, 2:4], dec4[:, 2:4].bitcast(F32),
                                enc4[:, 2:4].bitcast(F32), OP.subtract)
                        else:
                            nc.vector.tensor_tensor(D4t[:],
                                                    dec4[:].bitcast(F32),
                                                    enc4[:].bitcast(F32),
                                                    OP.subtract)
                        return ("sbuf", D4t)
                    D4ap = psd.tile([128, T, 512], F32)
                    for t in range(T):
                        nc.tensor.matmul(out=D4ap[:, t, 0:BL],
                                         lhsT=idp_t, rhs=dec4[:, t],
                                         start=True, stop=False)
                        nc.tensor.matmul(out=D4ap[:, t, 0:BL],
                                         lhsT=idm_t, rhs=enc4[:, t],
                                         start=False, stop=True)
                    return ("psum", D4ap)

                def emit_gD_F(p, D4pack):
                    # gD = (1+h)*0.5*(dec-enc) = sigma(pre)*(dec-enc)
                    kind, D4ap = D4pack
                    if kind == "sbuf":
                        # D unscaled: fold the 0.5 into F's scalar instead
                        D4v, fscale = D4ap[:], 0.5
                    else:
                        D4v, fscale = D4ap[:, :, 0:BL], 1.0
                    gD4 = wkg.tile([128, T, BL], F32)
                    F4 = wkf.tile([128, T, BL], F32)
                    if p == 0:
                        for sl in (slice(0, 2), slice(2, 4)):
                            nc.vector.scalar_tensor_tensor(
                                out=gD4[:, sl], in0=h4s[p][:, sl],
                                scalar=1.0, in1=D4ap[:, sl],
                                op0=OP.add, op1=OP.mult)
                            nc.vector.scalar_tensor_tensor(
                                out=F4[:, sl], in0=gD4[:, sl],
                                scalar=fscale,
                                in1=enc4s[p][:, sl].bitcast(F32),
                                op0=OP.mult, op1=OP.add)
                    else:
                        nc.vector.scalar_tensor_tensor(out=gD4[:],
                                                       in0=h4s[p][:],
                                                       scalar=1.0, in1=D4v,
                                                       op0=OP.add,
                                                       op1=OP.mult)
                        nc.vector.scalar_tensor_tensor(
                            out=F4[:], in0=gD4[:], scalar=fscale,
                            in1=enc4s[p][:].bitcast(F32),
                            op0=OP.mult, op1=OP.add)
                    F4s[p] = F4
                    del h4s[p]
                    del dec4s[p], enc4s[p]

                def emit_lif(p):
                    F4 = F4s[p]
                    vp = F4[:, 0]
                    vlist = [vp]
                    for t in range(T - 1):
                        vrn = vv.tile([128, BL], F32)
                        nc.vector.scalar_tensor_tensor(
                            out=vrn[:], in0=vp, scalar=TH, in1=vp,
                            op0=OP.is_lt, op1=OP.mult)
                        vpt = sm.tile([128, BL], F32)
                        nc.vector.scalar_tensor_tensor(
                            out=vpt[:], in0=vrn[:], scalar=0.5,
                            in1=F4[:, t + 1], op0=OP.mult, op1=OP.add)
                        vp = vpt[:]
                        vlist.append(vp)
                    vps[p] = vlist

                D4prev = {}
                for k in range(NPAIR + 4):
                    if k >= 3:
                        emit_signs(k - 4)   # signs for pair k-4 (t>=1)
                                            # and pair k-3 (t=0)
                    if k < NPAIR:
                        emit_dma(k)
                    if 0 <= k - 1 < NPAIR:
                        emit_pre_h(k - 1)
                        D4prev[k - 1] = emit_D(k - 1)
                    if 0 <= k - 2 < NPAIR:
                        emit_gD_F(k - 2, D4prev.pop(k - 2))
                    if 0 <= k - 3 < NPAIR:
                        emit_lif(k - 3)

                # ---------------- stats ----------------
                # per-channel sign sum: contract the two 64-partition halves
                # of scol with a stacked identity on PE, then reduce tiles
                # per-channel sign sum duplicated on both partition
                # halves via one matmul with a [2,2]-tiled identity; lands
                # in the last pair's (long since consumed) P4 bank
                scolR = cp.tile([128, NTILE], F32R)
                nc.vector.tensor_scalar(out=scolR[:], in0=scol[:],
                                        scalar1=1.0, scalar2=None,
                                        op0=OP.mult)
                ssum = P4s[NPAIR - 1][:, 0, 0:NTILE]
                nc.tensor.matmul(out=ssum, lhsT=i2x_t,
                                 rhs=scolR[:],
                                 start=True, stop=True)
                s128 = cp.tile([128, 1], F32)
                nc.vector.tensor_reduce(out=s128[:], in_=ssum,
                                        axis=AX.X, op=OP.add)
                mu = cp.tile([128, 1], F32)
                if use_collective:
                    # local spike count = 0.5*sum_sign + N_CORE/2
                    loc = cp.tile([64, 1], F32)
                    nc.vector.tensor_scalar(out=loc[:], in0=s128[0:64, :],
                                            scalar1=0.5,
                                            scalar2=float(N_CORE) / 2.0,
                                            op0=OP.mult, op1=OP.add)
                    cin = dp.tile([64, 1], F32)
                    cout = dp.tile([64, 1], F32)
                    nc.sync.dma_start(cin[:], loc[:])
                    nc.gpsimd.collective_compute(
                        "AllReduce", OP.add,
                        replica_groups=[list(range(num_devices))],
                        ins=[cin.opt()], outs=[cout.opt()])
                    S128 = cp.tile([128, 1], F32)
                    nc.sync.dma_start(S128[0:64, :], cout[:])
                    nc.gpsimd.dma_start(S128[64:128, :], cout[:])
                    nc.vector.tensor_scalar(out=mu[:], in0=S128[:],
                                            scalar1=1.0 / float(N_TOTAL),
                                            scalar2=None, op0=OP.mult)
                else:
                    # mu = ((0.5*sum + N_CORE/2) * NCORES) / N_TOTAL
                    nc.vector.tensor_scalar(
                        out=mu[:], in0=s128[:],
                        scalar1=0.5 * NCORES / float(N_TOTAL),
                        scalar2=N_CORE * 0.5 * NCORES / float(N_TOTAL),
                        op0=OP.mult, op1=OP.add)
                # x = mu*(1-mu) + eps
                m1 = cp.tile([128, 1], F32)
                nc.vector.tensor_scalar(out=m1[:], in0=mu[:], scalar1=-1.0,
                                        scalar2=1.0, op0=OP.mult, op1=OP.add)
                x = cp.tile([128, 1], F32)
                nc.vector.tensor_tensor(x[:], m1[:], mu[:], OP.mult)
                nc.vector.tensor_scalar(out=x[:], in0=x[:], scalar1=EPS,
                                        scalar2=None, op0=OP.add)
                # r = 1/sqrt(x) + one Newton step r *= 1.5-0.5*x*r^2
                sq = cp.tile([128, 1], F32)
                nc.scalar.activation(sq[:], x[:], AF.Sqrt)
                r0 = cp.tile([128, 1], F32)
                nc.vector.reciprocal(r0[:], sq[:])
                e = cp.tile([128, 1], F32)
                nc.vector.tensor_tensor(e[:], r0[:], r0[:], OP.mult)
                nc.vector.tensor_tensor(e[:], e[:], x[:], OP.mult)
                nc.vector.tensor_scalar(out=e[:], in0=e[:], scalar1=-0.5,
                                        scalar2=1.5, op0=OP.mult, op1=OP.add)
                r = cp.tile([128, 1], F32)
                nc.vector.tensor_tensor(r[:], r0[:], e[:], OP.mult)
                # a = gamma*r ; scale = a/2 ; bias = a/2 + beta - mu*a
                a = cp.tile([128, 1], F32)
                nc.vector.tensor_tensor(a[:], gam_t, r[:], OP.mult)
                sc128 = cp.tile([128, 1], F32)
                nc.vector.tensor_scalar(out=sc128[:], in0=a[:], scalar1=0.5,
                                        scalar2=None, op0=OP.mult)
                tmp = cp.tile([128, 1], F32)
                nc.vector.tensor_tensor(tmp[:], mu[:], a[:], OP.mult)
                b0 = cp.tile([128, 1], F32)
                nc.vector.tensor_tensor(b0[:], bet_t, tmp[:], OP.subtract)
                bi128 = cp.tile([128, 1], F32)
                nc.vector.tensor_tensor(bi128[:], sc128[:], b0[:], OP.add)

                # ---------------- pass 2 ----------------
                for pair in range(NPAIR):
                    ot = op_.tile([128, T, BL], F16)
                    nc.vector.tensor_scalar(
                        out=ot[:], in0=store[:, pair * T:(pair + 1) * T, :],
                        scalar1=sc128[:], scalar2=bi128[:],
                        op0=OP.mult, op1=OP.add)
                    eng = (nc.sync, nc.gpsimd, nc.scalar)[pair % 3]
                    eng.dma_start(out_d[pair], ot[:])

    nc.compile()
    return nc


def _prep_host(dec, enc, Wg, bg, gamma, beta):
    Wg = np.asarray(Wg, dtype=np.float32)
    wdT = np.ascontiguousarray(Wg[:, :64].T)   # [k, m] dec-part
    weT = np.ascontiguousarray(Wg[:, 64:].T)   # enc-part
    wd = np.zeros((128, 128), dtype=np.float32)
    we = np.zeros((128, 128), dtype=np.float32)
    wd[:64, :64] = wdT
    wd[64:, 64:] = wdT
    we[:64, :64] = weT
    we[64:, 64:] = weT
    bgh = np.tile(0.5 * np.asarray(bg, np.float32), 2)
    idp = np.eye(128, dtype=np.float32) * 0.5
    idm = np.eye(128, dtype=np.float32) * -0.5

    def relayout(x):
        # [T, C, NPIX] -> [pair, p2*64+c, t, x448]
        x = np.asarray(x, np.float32).reshape(T, C, NPAIR, 2, BL)
        return np.ascontiguousarray(x.transpose(2, 3, 1, 0, 4)
                                    .reshape(NPAIR, 128, T, BL))
    par = np.zeros((128, 644), dtype=np.float32)
    par[:, 0:128] = wd
    par[:, 128:256] = we
    par[:, 256:384] = idp
    par[:, 384:512] = idm
    par[:, 512] = bgh
    par[:, 513] = -TH
    par[:, 514] = np.tile(np.asarray(gamma, np.float32), 2)
    par[:, 515] = np.tile(np.asarray(beta, np.float32), 2)
    par[:, 516:644] = np.tile(np.eye(64, dtype=np.float32), (2, 2))
    in_maps = []
    for b in range(NCORES):
        in_maps.append({
            "dec": relayout(np.asarray(dec[:, b]).reshape(T, C, NPIX)),
            "enc": relayout(np.asarray(enc[:, b]).reshape(T, C, NPIX)),
            "par": par,
        })
    return in_maps


def kernel(dec, enc, Wg, bg, gamma, beta, _trace=False, _trace_kwargs=None):
    from concourse.bass_utils import run_bass_kernel_spmd

    if "nc" not in _cache:
        _cache["nc"] = _build()
    nc = _cache["nc"]

    in_maps = _prep_host(dec, enc, Wg, bg, gamma, beta)
    kw = {}
    if _trace:
        kw["trace"] = True
        if _trace_kwargs:
            kw.update(_trace_kwargs)
    res = run_bass_kernel_spmd(nc, in_maps, core_ids=list(range(NCORES)), **kw)
    outs = []
    for b in range(NCORES):
        o = np.asarray(res.results[b]["out"]).astype(np.float32)
        # [pair, p2*64+c, t, x448] -> [T, C, NPIX]
        o = o.reshape(NPAIR, 2, C, T, BL).transpose(3, 2, 0, 1, 4)
        outs.append(o.reshape(T, C, NPIX))
    out = np.stack(outs, axis=1).reshape(T, B, C, H, W)
    if _trace:
        _cache["last_res"] = res
    return out
